# revision 6
# baseline (speedup 1.0000x reference)
"""Conv2d(1->16,5x5,p2) + BN(inference) + ReLU + MaxPool2d(2) on 8 NeuronCores.

Strategy (per core, 16 images = data parallelism over batch):
  - BN is folded into the conv weights/bias on the host.
  - Conv is computed on the TensorEngine as a single matmul per 16-output-row
    slab: contraction K = (dx-block j in 0..4) x (input row yi in 0..19) = 100.
    The 5 dx shifts are materialized as 5 partition-blocks of the slab tile,
    loaded directly from HBM with column offset j (overlapping reads).
    The dy taps are encoded in a Toeplitz weight matrix lhsT[(j,yi), (o,yp)].
  - Two matmuls per slab produce even / odd output rows in separate PSUM
    banks; the 2x2 maxpool becomes: vertical max = elementwise max of the
    two PSUM tiles (DVE), horizontal max = strided max in SBUF.
  - The run is wire-bound through the axon tunnel (~40-50 MB/s), so I/O is
    compressed: x is uploaded as int8 (one global scale, folded into the
    bf16 weights; GpSimd casts i8->bf16 on device), and the output is
    downloaded as uint8 quantized per partition row (per (channel, out-row)
    x image-pair) with f32 dequant scales as a tiny side output. ReLU +
    folded bias + quantization run fused on the ScalarEngine; dequant
    happens on host. This cuts wire bytes ~4x vs f32 in/out (232 -> 60 MB).
"""

import time
from concurrent.futures import ThreadPoolExecutor

import numpy as np
import ml_dtypes

import concourse.bass as bass
import concourse.bacc as bacc
import concourse.tile as tile
import concourse.mybir as mybir
from concourse.bass_utils import run_bass_kernel_spmd

F32 = mybir.dt.float32
BF16 = mybir.dt.bfloat16
U8 = mybir.dt.uint8
I8 = mybir.dt.int8
N_CORES = 8
B, H, W = 128, 224, 224
PB = B // N_CORES          # images per core
PH, PW = H + 4, W + 4      # host-padded image
OC = 16
HO, WO = H // 2, W // 2    # 112, 112
YB = 16                    # conv output rows per slab
NT = H // YB               # 14 slabs per image pair
NPAIR = PB // 2            # image pairs per core
NIT = NPAIR * NT           # 112 iterations -> scale columns
KROWS = YB + 4             # input rows per dx-block
K = 5 * KROWS              # 100 contraction partitions
BN_EPS = 1e-5
QMAX = 254.0               # u8 quant full-scale (0.5 headroom below 255)

_CACHE: dict = {}


def _build_nc():
    nc = bacc.Bacc("TRN2", num_devices=N_CORES)
    xpad = nc.dram_tensor("xpad", [PB, PH, PW], I8, kind="ExternalInput")
    lhsE_d = nc.dram_tensor("lhsE", [K, 128], BF16, kind="ExternalInput")
    lhsO_d = nc.dram_tensor("lhsO", [K, 128], BF16, kind="ExternalInput")
    bias_d = nc.dram_tensor("bias", [128, 1], F32, kind="ExternalInput")
    out = nc.dram_tensor("out", [PB, OC, HO, WO], U8, kind="ExternalOutput")
    sc_d = nc.dram_tensor("sc", [128, NIT], F32, kind="ExternalOutput")

    with tile.TileContext(nc) as tc:
        with (
            tc.tile_pool(name="const", bufs=1) as constp,
            tc.tile_pool(name="s8", bufs=4) as s8p,
            tc.tile_pool(name="s", bufs=4) as sp,
            tc.tile_pool(name="v", bufs=3) as vp,
            tc.tile_pool(name="h", bufs=3) as hp,
            tc.tile_pool(name="q", bufs=3) as qp,
            tc.tile_pool(name="r", bufs=3) as rp,
            tc.tile_pool(name="ps", bufs=4, space="PSUM") as pp,
        ):
            lE = constp.tile([K, 128], BF16, tag="lE")
            nc.sync.dma_start(lE[:], lhsE_d.ap())
            lO = constp.tile([K, 128], BF16, tag="lO")
            nc.sync.dma_start(lO[:], lhsO_d.ap())
            bt = constp.tile([128, 1], F32, tag="bias")
            nc.sync.dma_start(bt[:], bias_d.ap())
            Sc = constp.tile([128, NIT], F32, tag="Sc")

            for pi in range(NPAIR):         # image pairs
                for t in range(NT):         # y slabs
                    it = pi * NT + t
                    y0 = YB * t
                    S8 = s8p.tile([K, 448], I8, tag="S8")
                    for i in range(2):
                        src = bass.AP(
                            xpad,
                            (2 * pi + i) * PH * PW + y0 * PW,
                            [[1, 5], [PW, KROWS], [1, 224]],
                        )
                        nc.sync.dma_start(S8[:, i * 224:(i + 1) * 224], src)
                    # GpSimd casts the slab to bf16 for the TensorEngine
                    S = sp.tile([K, 448], BF16, tag="S")
                    nc.gpsimd.tensor_copy(S[:], S8[:])

                    pe_t = pp.tile([128, 448], F32, tag="ps")
                    nc.tensor.matmul(pe_t[:], lE[:], S[:], start=True, stop=True)
                    po_t = pp.tile([128, 448], F32, tag="ps")
                    nc.tensor.matmul(po_t[:], lO[:], S[:], start=True, stop=True)

                    # ACT drains the odd bank to SBUF (DVE cannot read two
                    # PSUM streams in one tensor_tensor)
                    CO = vp.tile([128, 448], F32, tag="CO")
                    nc.scalar.copy(CO[:], po_t[:])
                    # vertical max: PSUM + SBUF operands
                    V = vp.tile([128, 448], F32, tag="V")
                    nc.vector.tensor_max(V[:], pe_t[:], CO[:])
                    # horizontal max: strided SBUF (pre-bias, pre-ReLU pooled)
                    Hm = hp.tile([128, 224], F32, tag="H")
                    v4 = V[:].rearrange("p (i xp two) -> p i xp two", i=2, two=2)
                    h3 = Hm[:].rearrange("p (i xp) -> p i xp", i=2)
                    nc.vector.tensor_max(h3, v4[:, :, :, 0], v4[:, :, :, 1])

                    # Quantization scales: row max of ReLU(Hm + bias), i.e.
                    # max(rowmax(Hm) + bias, eps); dequant scale = that / QMAX.
                    M = rp.tile([128, 1], F32, tag="M")
                    nc.vector.reduce_max(M[:], Hm[:], axis=mybir.AxisListType.X)
                    t1 = rp.tile([128, 1], F32, tag="t1")
                    nc.vector.tensor_scalar(
                        out=t1[:], in0=M[:], scalar1=bt[:, 0:1], scalar2=1e-30,
                        op0=mybir.AluOpType.add, op1=mybir.AluOpType.max,
                    )
                    nc.vector.tensor_scalar_mul(Sc[:, it:it + 1], t1[:], 1.0 / QMAX)
                    R = rp.tile([128, 1], F32, tag="R")
                    nc.vector.reciprocal(R[:], Sc[:, it:it + 1])
                    B2 = rp.tile([128, 1], F32, tag="B2")
                    nc.vector.tensor_scalar(
                        out=B2[:], in0=bt[:, 0:1], scalar1=R[:, 0:1], scalar2=0.5,
                        op0=mybir.AluOpType.mult, op1=mybir.AluOpType.add,
                    )
                    # q = u8(Relu(Hm * R + (bias*R + 0.5))) fused on ScalarE
                    Q = qp.tile([128, 224], U8, tag="Q")
                    nc.scalar.activation(
                        Q[:], Hm[:], mybir.ActivationFunctionType.Relu,
                        bias=B2[:, 0:1], scale=R[:, 0:1],
                    )

                    for i in range(2):
                        dst = bass.AP(
                            out,
                            (2 * pi + i) * OC * HO * WO + (8 * t) * WO,
                            [[HO * WO, OC], [WO, 8], [1, WO]],
                        )
                        nc.scalar.dma_start(dst, Q[:, i * WO:(i + 1) * WO])

            nc.sync.dma_start(sc_d.ap(), Sc[:])

    nc.compile()
    return nc


def _host_prep(x, conv_w, conv_b, gamma, beta, run_mean, run_var):
    scale = (gamma / np.sqrt(run_var + BN_EPS)).astype(np.float32)
    wf = (conv_w[:, 0] * scale[:, None, None]).astype(np.float32)       # [16,5,5]
    bf = (conv_b * scale + beta - run_mean * scale).astype(np.float32)  # [16]

    # int8-quantize x with one global scale; fold the scale into the weights
    x = x.reshape(B, H, W)
    sx = float(np.abs(x).max()) / 127.0
    if sx == 0.0:
        sx = 1.0
    tmp = np.multiply(x, 1.0 / sx)
    np.rint(tmp, out=tmp)
    xpad = np.zeros((B, PH, PW), np.int8)
    xpad[:, 2:2 + H, 2:2 + W] = tmp  # integral-valued f32 -> exact i8 cast
    wf *= sx

    lhsE = np.zeros((K, 128), np.float32)
    lhsO = np.zeros((K, 128), np.float32)
    bias = np.zeros((128, 1), np.float32)
    for o in range(OC):
        for yp in range(8):
            m = o * 8 + yp
            bias[m, 0] = bf[o]
            for j in range(5):
                for dy in range(5):
                    lhsE[j * KROWS + 2 * yp + dy, m] = wf[o, dy, j]
                    lhsO[j * KROWS + 2 * yp + 1 + dy, m] = wf[o, dy, j]

    return (xpad, lhsE.astype(ml_dtypes.bfloat16),
            lhsO.astype(ml_dtypes.bfloat16), bias)


def kernel(x, conv_w, conv_b, gamma, beta, run_mean, run_var, _trace=False):
    t0 = time.perf_counter()
    x = np.asarray(x, np.float32)
    conv_w = np.asarray(conv_w, np.float32)
    conv_b = np.asarray(conv_b, np.float32)
    gamma = np.asarray(gamma, np.float32)
    beta = np.asarray(beta, np.float32)
    run_mean = np.asarray(run_mean, np.float32)
    run_var = np.asarray(run_var, np.float32)
    xpad, lhsE, lhsO, bias = _host_prep(
        x, conv_w, conv_b, gamma, beta, run_mean, run_var
    )
    t1 = time.perf_counter()
    if "nc" not in _CACHE:
        _CACHE["nc"] = _build_nc()
    nc = _CACHE["nc"]
    t2 = time.perf_counter()
    in_maps = [
        {
            "xpad": xpad[c * PB:(c + 1) * PB],
            "lhsE": lhsE,
            "lhsO": lhsO,
            "bias": bias,
        }
        for c in range(N_CORES)
    ]
    res = run_bass_kernel_spmd(nc, in_maps, core_ids=list(range(N_CORES)),
                               trace=_trace)
    t3 = time.perf_counter()

    # Host-side dequant: out[img, o, 8t+yp, :] = u8 * sc[o*8+yp, pi*NT+t]
    # for img = 2*pi + i (both images of a pair share the scale column).
    out = np.empty((B, OC, HO, WO), np.float32)

    def _dequant(c):
        r = res.results[c]
        s = r["sc"].reshape(OC, 8, NPAIR, NT)                   # [o, yp, pi, t]
        s_img = s.transpose(2, 0, 3, 1).reshape(NPAIR, OC, HO)  # [pi, o, row]
        u8 = r["out"].reshape(NPAIR, 2, OC, HO, WO)
        dst = out[c * PB:(c + 1) * PB].reshape(NPAIR, 2, OC, HO, WO)
        np.multiply(u8, s_img[:, None, :, :, None], out=dst, casting="unsafe")

    with ThreadPoolExecutor(N_CORES) as ex:
        list(ex.map(_dequant, range(N_CORES)))
    t4 = time.perf_counter()
    _CACHE["last_results"] = res
    _CACHE["timing"] = {
        "host_prep": t1 - t0, "build": t2 - t1,
        "spmd": t3 - t2, "dequant": t4 - t3,
    }
    return out


# revision 10
# speedup vs baseline: 1.0458x; 1.0458x over previous
"""Conv2d(1->16,5x5,p2) + BN(inference) + ReLU + MaxPool2d(2) on 8 NeuronCores.

Strategy (per core, 16 images = data parallelism over batch):
  - BN is folded into the conv weights/bias on the host.
  - Conv is computed on the TensorEngine as a single matmul per 16-output-row
    slab: contraction K = (dx-block j in 0..4) x (input row yi in 0..19) = 100.
    The 5 dx shifts are materialized as 5 partition-blocks of the slab tile,
    loaded directly from HBM with column offset j (overlapping reads).
    The dy taps are encoded in a Toeplitz weight matrix lhsT[(j,yi), (o,yp)].
  - Two matmuls per slab produce even / odd output rows in separate PSUM
    banks; the 2x2 maxpool becomes: vertical max = elementwise max of the
    two PSUM tiles (DVE), horizontal max = strided max in SBUF.
  - The run is wire-bound through the axon tunnel (~40-50 MB/s), so I/O is
    compressed: x is uploaded as int8 (one global scale, folded into the
    bf16 weights; GpSimd casts i8->bf16 on device), and the output is
    downloaded as uint8 quantized per partition row (per (channel, out-row)
    x image-pair) with f32 dequant scales bit-packed into the tail of the
    same u8 blob. ReLU + folded bias + quantization run fused on the
    ScalarEngine; dequant happens on host. This cuts wire bytes ~4x vs
    f32 in/out (232 -> 60 MB). I/O is packed into 2 input + 1 output
    arrays to minimize per-array transfer round trips.
"""

import time
from concurrent.futures import ThreadPoolExecutor

import numpy as np
import ml_dtypes

import concourse.bass as bass
import concourse.bacc as bacc
import concourse.tile as tile
import concourse.mybir as mybir
from concourse.bass_utils import run_bass_kernel_spmd

F32 = mybir.dt.float32
BF16 = mybir.dt.bfloat16
U8 = mybir.dt.uint8
I8 = mybir.dt.int8
N_CORES = 8
B, H, W = 128, 224, 224
PB = B // N_CORES          # images per core
PH, PW = H + 4, W + 4      # host-padded image
OC = 16
HO, WO = H // 2, W // 2    # 112, 112
YB = 16                    # conv output rows per slab
NT = H // YB               # 14 slabs per image pair
NPAIR = PB // 2            # image pairs per core
NIT = NPAIR * NT           # 112 iterations -> scale columns
KROWS = YB + 4             # input rows per dx-block
K = 5 * KROWS              # 100 contraction partitions
BN_EPS = 1e-5
QMAX = 254.0               # u8 quant full-scale (0.5 headroom below 255)
QOFF = PB * OC * HO * WO   # u8 quant bytes per core; scales live after
SCB = NIT * 4              # scale bytes per partition row
OUTB = QOFF + 128 * SCB    # total output blob bytes per core

_CACHE: dict = {}


def _build_nc():
    nc = bacc.Bacc("TRN2", num_devices=N_CORES)
    xpad = nc.dram_tensor("xpad", [PB, PH, PW], I8, kind="ExternalInput")
    # packed consts: cols 0:128 lhsE, 128:256 lhsO (K=100 rows used),
    # col 256 = folded bias (all 128 rows)
    const_d = nc.dram_tensor("const", [128, 257], BF16, kind="ExternalInput")
    out = nc.dram_tensor("out", [OUTB], U8, kind="ExternalOutput")

    with tile.TileContext(nc) as tc:
        with (
            tc.tile_pool(name="const", bufs=1) as constp,
            tc.tile_pool(name="s8", bufs=4) as s8p,
            tc.tile_pool(name="s", bufs=4) as sp,
            tc.tile_pool(name="v", bufs=3) as vp,
            tc.tile_pool(name="h", bufs=3) as hp,
            tc.tile_pool(name="q", bufs=3) as qp,
            tc.tile_pool(name="r", bufs=3) as rp,
            tc.tile_pool(name="ps", bufs=4, space="PSUM") as pp,
        ):
            CT = constp.tile([128, 257], BF16, tag="CT")
            nc.sync.dma_start(CT[:], const_d.ap())
            lE = CT[0:K, 0:128]
            lO = CT[0:K, 128:256]
            # per-partition scalar operands must be f32: cast bias once
            btf = constp.tile([128, 1], F32, tag="btf")
            nc.vector.tensor_copy(btf[:], CT[:, 256:257])
            bt = btf[:, 0:1]
            Sc = constp.tile([128, NIT], F32, tag="Sc")

            for pi in range(NPAIR):         # image pairs
                for t in range(NT):         # y slabs
                    it = pi * NT + t
                    y0 = YB * t
                    S8 = s8p.tile([K, 448], I8, tag="S8")
                    for i in range(2):
                        src = bass.AP(
                            xpad,
                            (2 * pi + i) * PH * PW + y0 * PW,
                            [[1, 5], [PW, KROWS], [1, 224]],
                        )
                        nc.sync.dma_start(S8[:, i * 224:(i + 1) * 224], src)
                    # GpSimd casts the slab to bf16 for the TensorEngine
                    S = sp.tile([K, 448], BF16, tag="S")
                    nc.gpsimd.tensor_copy(S[:], S8[:])

                    pe_t = pp.tile([128, 448], F32, tag="ps")
                    nc.tensor.matmul(pe_t[:], lE, S[:], start=True, stop=True)
                    po_t = pp.tile([128, 448], F32, tag="ps")
                    nc.tensor.matmul(po_t[:], lO, S[:], start=True, stop=True)

                    # ACT drains the odd bank to SBUF (DVE cannot read two
                    # PSUM streams in one tensor_tensor)
                    CO = vp.tile([128, 448], F32, tag="CO")
                    nc.scalar.copy(CO[:], po_t[:])
                    # vertical max: PSUM + SBUF operands
                    V = vp.tile([128, 448], F32, tag="V")
                    nc.vector.tensor_max(V[:], pe_t[:], CO[:])
                    # horizontal max: strided SBUF (pre-bias, pre-ReLU pooled)
                    Hm = hp.tile([128, 224], F32, tag="H")
                    v4 = V[:].rearrange("p (i xp two) -> p i xp two", i=2, two=2)
                    h3 = Hm[:].rearrange("p (i xp) -> p i xp", i=2)
                    nc.vector.tensor_max(h3, v4[:, :, :, 0], v4[:, :, :, 1])

                    # Quantization scales: row max of ReLU(Hm + bias), i.e.
                    # max(rowmax(Hm) + bias, eps); dequant scale = that / QMAX.
                    M = rp.tile([128, 1], F32, tag="M")
                    nc.vector.reduce_max(M[:], Hm[:], axis=mybir.AxisListType.X)
                    t1 = rp.tile([128, 1], F32, tag="t1")
                    nc.vector.tensor_scalar(
                        out=t1[:], in0=M[:], scalar1=bt, scalar2=1e-30,
                        op0=mybir.AluOpType.add, op1=mybir.AluOpType.max,
                    )
                    nc.vector.tensor_scalar_mul(Sc[:, it:it + 1], t1[:], 1.0 / QMAX)
                    R = rp.tile([128, 1], F32, tag="R")
                    nc.vector.reciprocal(R[:], Sc[:, it:it + 1])
                    B2 = rp.tile([128, 1], F32, tag="B2")
                    nc.vector.tensor_scalar(
                        out=B2[:], in0=bt, scalar1=R[:, 0:1], scalar2=0.5,
                        op0=mybir.AluOpType.mult, op1=mybir.AluOpType.add,
                    )
                    # q = u8(Relu(Hm * R + (bias*R + 0.5))) fused on ScalarE
                    Q = qp.tile([128, 224], U8, tag="Q")
                    nc.scalar.activation(
                        Q[:], Hm[:], mybir.ActivationFunctionType.Relu,
                        bias=B2[:, 0:1], scale=R[:, 0:1],
                    )

                    for i in range(2):
                        dst = bass.AP(
                            out,
                            (2 * pi + i) * OC * HO * WO + (8 * t) * WO,
                            [[HO * WO, OC], [WO, 8], [1, WO]],
                        )
                        nc.scalar.dma_start(dst, Q[:, i * WO:(i + 1) * WO])

            # scale bytes appended to the u8 blob (f32 bit-packed)
            scdst = bass.AP(out, QOFF, [[SCB, 128], [1, SCB]])
            nc.sync.dma_start(scdst, Sc[:].bitcast(U8))

    nc.compile()
    return nc


def _host_prep(x, conv_w, conv_b, gamma, beta, run_mean, run_var):
    scale = (gamma / np.sqrt(run_var + BN_EPS)).astype(np.float32)
    wf = (conv_w[:, 0] * scale[:, None, None]).astype(np.float32)       # [16,5,5]
    bf = (conv_b * scale + beta - run_mean * scale).astype(np.float32)  # [16]

    # int8-quantize x with one global scale; fold the scale into the weights
    x = x.reshape(B, H, W)
    sx = float(np.abs(x).max()) / 127.0
    if sx == 0.0:
        sx = 1.0
    tmp = np.multiply(x, 1.0 / sx)
    np.rint(tmp, out=tmp)
    xpad = np.zeros((B, PH, PW), np.int8)
    xpad[:, 2:2 + H, 2:2 + W] = tmp  # integral-valued f32 -> exact i8 cast
    wf *= sx

    lhsE = np.zeros((K, 128), np.float32)
    lhsO = np.zeros((K, 128), np.float32)
    for o in range(OC):
        for yp in range(8):
            m = o * 8 + yp
            for j in range(5):
                for dy in range(5):
                    lhsE[j * KROWS + 2 * yp + dy, m] = wf[o, dy, j]
                    lhsO[j * KROWS + 2 * yp + 1 + dy, m] = wf[o, dy, j]

    const = np.zeros((128, 257), ml_dtypes.bfloat16)
    const[0:K, 0:128] = lhsE
    const[0:K, 128:256] = lhsO
    const[:, 256] = np.repeat(bf, 8)  # bias per partition row (o*8+yp)
    return xpad, const


def kernel(x, conv_w, conv_b, gamma, beta, run_mean, run_var, _trace=False):
    t0 = time.perf_counter()
    x = np.asarray(x, np.float32)
    conv_w = np.asarray(conv_w, np.float32)
    conv_b = np.asarray(conv_b, np.float32)
    gamma = np.asarray(gamma, np.float32)
    beta = np.asarray(beta, np.float32)
    run_mean = np.asarray(run_mean, np.float32)
    run_var = np.asarray(run_var, np.float32)
    xpad, const = _host_prep(
        x, conv_w, conv_b, gamma, beta, run_mean, run_var
    )
    t1 = time.perf_counter()
    if "nc" not in _CACHE:
        _CACHE["nc"] = _build_nc()
    nc = _CACHE["nc"]
    t2 = time.perf_counter()
    in_maps = [
        {"xpad": xpad[c * PB:(c + 1) * PB], "const": const}
        for c in range(N_CORES)
    ]
    res = run_bass_kernel_spmd(nc, in_maps, core_ids=list(range(N_CORES)),
                               trace=_trace)
    t3 = time.perf_counter()

    # Host-side dequant: out[img, o, 8t+yp, :] = u8 * sc[o*8+yp, pi*NT+t]
    # for img = 2*pi + i (both images of a pair share the scale column).
    out = np.empty((B, OC, HO, WO), np.float32)

    def _dequant(c):
        blob = res.results[c]["out"]
        sc = blob[QOFF:].view(np.float32).reshape(128, NIT)
        s = sc.reshape(OC, 8, NPAIR, NT)                        # [o, yp, pi, t]
        s_img = s.transpose(2, 0, 3, 1).reshape(NPAIR, OC, HO)  # [pi, o, row]
        u8 = blob[:QOFF].reshape(NPAIR, 2, OC, HO, WO)
        dst = out[c * PB:(c + 1) * PB].reshape(NPAIR, 2, OC, HO, WO)
        np.multiply(u8, s_img[:, None, :, :, None], out=dst, casting="unsafe")

    with ThreadPoolExecutor(N_CORES) as ex:
        list(ex.map(_dequant, range(N_CORES)))
    t4 = time.perf_counter()
    _CACHE["last_results"] = res
    _CACHE["timing"] = {
        "host_prep": t1 - t0, "build": t2 - t1,
        "spmd": t3 - t2, "dequant": t4 - t3,
    }
    return out
